# revision 17
# baseline (speedup 1.0000x reference)
"""BERT attention block (QKV -> MHA -> output proj -> residual -> LayerNorm)
on 8 Trainium2 NeuronCores.

Sharding: data parallel over (batch, query-half). Core c handles batch b=c//2
and query rows [half*1024, (half+1)*1024) of that batch element (half=c%2).
Each core computes K/V for the full 2048-token sequence of its batch element
(duplicated across the 2 cores sharing a batch element), so no collectives
are needed. The per-core difference is entirely in the data (SPMD program).

All matmul operands are fp8 (e4m3, fp32 accumulation in PSUM), with DoubleRow
perf mode (two fp8 weights per PE cell -> contraction 256 per pass) on every
128-contraction matmul: the QKV/output projections pair adjacent hidden-dim
chunks, the ctx matmuls pair adjacent key tiles. Scores matmuls (contraction
64 per head) stay in normal mode with two heads packed via PE row groups.

The attention loop is paced by the scalar engine's exp throughput, so the
emission order keeps it saturated: per head pair, BOTH query chunks' scores
+exp run back-to-back (exp pool holds 4 half-tiles so the activation never
waits on ctx consumers), and the resulting ~33us exp window hides the V
projection (pair 0), the next pair's K/Q projection, both ctx accumulations,
and - on the last pair - the first half of the epilogue. The epilogue for
query rows 0-511 is emitted inside pair 7's qc=1 exp window (it only needs
qc=0's ctx columns), so only rows 512-1023 epilogue remains as tail.

fp8 scaling: x is prescaled x2 and weights x4 on the host so the weight
values clear e4m3's subnormal range. Q/K/V come out 8x true scale; scores
64x (folded into the exp scale); the softmax denominator's ones-column is
1/64 so ctx lands 512x its true value in fp8 (good range), and the output
projection descales by 1/2048 on the DVE. Softmax uses exp(s/8) with no max
subtraction (|s/8| is a few units at most for this distribution); the
attention mask folds in multiplicatively: V' rows (including the ones
column) are scaled by exp(mask[k]) after the V projection, so the exp
activation needs no per-key-tile bias and the denominator falls out of the
ctx matmul via V's scaled ones column (row 64 of the ctx accumulator).

The residual + LayerNorm path is fp32 end to end.
"""

import numpy as np
import ml_dtypes

import concourse.bass as bass
import concourse.mybir as mybir
import concourse.tile as tile
from concourse import bacc

# Problem constants (hardcoded per the harness contract).
B = 4
S = 2048
H = 1024
NH = 16
HD = 64
EPS = 1e-12
N_CORES = 8
SQ = 1024  # query rows per core
P = 128
NJ = H // P      # 8 hidden-dim chunks
NKT = S // P     # 16 key tiles
NQC = SQ // 512  # 2 query chunks of 512
NTOK = SQ // P   # 8 query-row tiles
NPAIR = NH // 2  # 8 head pairs

F8 = mybir.dt.float8e4
F32 = mybir.dt.float32
BF16 = mybir.dt.bfloat16
NPF8 = ml_dtypes.float8_e4m3
DR = mybir.MatmulPerfMode.DoubleRow

XS = 2.0                 # host prescale on x
WS = 4.0                 # host prescale on all four weight matrices
QKV_S = XS * WS          # q/k/v tiles are 8x true scale
EXP_SCALE = 0.125 / (QKV_S * QKV_S)   # exp(s_true/8) from 64x-scaled scores
ONES_VAL = 1.0 / 64.0    # denominator column value -> ctx stored 512x true
OUT_DESCALE = 1.0 / (64.0 * XS * WS * WS)  # after ctx @ woT


def build_program():
    nc = bacc.Bacc("TRN2", target_bir_lowering=False, debug=False)

    xT = nc.dram_tensor("xT", [H, S], F8, kind="ExternalInput").ap()
    xqT = nc.dram_tensor("xqT", [H, SQ], F8, kind="ExternalInput").ap()
    xres = nc.dram_tensor("xres", [SQ, H], F32, kind="ExternalInput").ap()
    wqT = nc.dram_tensor("wqT", [H, H], F8, kind="ExternalInput").ap()
    wkT = nc.dram_tensor("wkT", [H, H], F8, kind="ExternalInput").ap()
    wvT = nc.dram_tensor("wvT", [H, H], F8, kind="ExternalInput").ap()
    woT = nc.dram_tensor("woT", [H, H], F8, kind="ExternalInput").ap()
    bq_c = nc.dram_tensor("bq_c", [P, NJ], F32, kind="ExternalInput").ap()
    bk_c = nc.dram_tensor("bk_c", [P, NJ], F32, kind="ExternalInput").ap()
    bv = nc.dram_tensor("bv", [H], F32, kind="ExternalInput").ap()
    gamma = nc.dram_tensor("gamma", [H], BF16, kind="ExternalInput").ap()
    beta = nc.dram_tensor("beta", [H], F32, kind="ExternalInput").ap()
    em_kt = nc.dram_tensor("em_kt", [P, NKT], F32, kind="ExternalInput").ap()
    y = nc.dram_tensor("y", [SQ, H], F32, kind="ExternalOutput").ap()

    with tile.TileContext(nc) as tc:
        _emit(tc, xT, xqT, xres, wqT, wkT, wvT, woT, bq_c, bk_c, bv,
              gamma, beta, em_kt, y)
    nc.compile()
    return nc


def _emit(tc, xT, xqT, xres, wqT, wkT, wvT, woT, bq_c, bk_c, bv, gamma,
          beta, em_kt, y):
    nc = tc.nc

    def bcast(v):  # [H] DRAM vector -> [P, H] partition-broadcast AP
        return bass.AP(tensor=v.tensor, offset=v.offset,
                       ap=[[0, P], list(v.ap[0])])

    def chunked(w):  # [H, N] DRAM -> [P, NJ, N]
        return w.rearrange("(j p) f -> p j f", p=P)

    with (
        tc.tile_pool(name="persist", bufs=1) as persist,
        tc.tile_pool(name="small", bufs=1) as small,
        tc.tile_pool(name="psProj", bufs=2, space="PSUM") as psProj,
        tc.tile_pool(name="epi", bufs=3) as epi,
        tc.tile_pool(name="stat", bufs=3) as stat,
    ):
        ctxT_sb = persist.tile([P, NJ, SQ], F8)  # 512x ctx.T [feat, tok]

        consts = small.tile([P, 2 * NJ + NKT + 1], F32)
        bq_sb = consts[:, 0:NJ]
        bk_sb = consts[:, NJ : 2 * NJ]
        em_sb = consts[:, 2 * NJ : 2 * NJ + NKT]
        eps_sb = consts[:, 2 * NJ + NKT :]
        nc.sync.dma_start(bq_sb, bq_c)
        nc.sync.dma_start(bk_sb, bk_c)
        nc.sync.dma_start(em_sb, em_kt)
        nc.vector.memset(eps_sb, EPS)
        gamma_b = small.tile([P, H], BF16)
        beta_b = small.tile([P, H], F32)
        woT_sb = persist.tile([P, NJ, H], F8)

        # ---- epilogue for one token tile: out proj + residual + LN ----
        def emit_epilogue_tile(tt):
            rs = slice(tt * P, (tt + 1) * P)
            x_t = epi.tile([P, H], F32, tag="x", name=f"x_{tt}")
            res_t = epi.tile([P, H], F32, tag="res", name=f"res_{tt}")
            y_t = epi.tile([P, H], F32, tag="y", name=f"y_{tt}")
            nc.sync.dma_start(res_t, xres[rs, :])
            for fc in range(2):
                fs = slice(fc * 512, (fc + 1) * 512)
                ps = psProj.tile([P, 512], F32, tag="psProj",
                                 name=f"psO_{tt}_{fc}")
                for j in range(NJ // 2):
                    nc.tensor.matmul(
                        ps,
                        lhsT=ctxT_sb[:, 2 * j : 2 * j + 2,
                                     tt * P : (tt + 1) * P],
                        rhs=woT_sb[:, 2 * j : 2 * j + 2, fs],
                        start=(j == 0),
                        stop=(j == NJ // 2 - 1),
                        perf_mode=DR,
                    )
                # bo is pre-folded into xres host-side, so one fused op:
                # x = ps*descale + (residual + bo).
                nc.vector.scalar_tensor_tensor(
                    out=x_t[:, fs], in0=ps, scalar=OUT_DESCALE,
                    in1=res_t[:, fs],
                    op0=mybir.AluOpType.mult, op1=mybir.AluOpType.add)
            st = stat.tile([P, 2, nc.vector.BN_STATS_DIM], F32, tag="st",
                           name=f"st_{tt}")
            mv = stat.tile([P, nc.vector.BN_AGGR_DIM], F32, tag="mv",
                           name=f"mv_{tt}")
            for g in range(2):
                nc.vector.bn_stats(out=st[:, g, :],
                                   in_=x_t[:, g * 512 : (g + 1) * 512])
            nc.vector.bn_aggr(out=mv, in_=st)
            sd = stat.tile([P, 1], F32, tag="sd", name=f"sd_{tt}")
            nc.scalar.activation(
                out=sd, in_=mv[:, 1:2],
                func=mybir.ActivationFunctionType.Sqrt,
                bias=eps_sb, scale=1.0,
            )
            rstd = stat.tile([P, 1], F32, tag="rstd", name=f"rstd_{tt}")
            nc.vector.reciprocal(rstd, sd)
            # x*rstd + (-mean*rstd) == (x - mean) * rstd, on the ACT engine.
            nmu = stat.tile([P, 1], F32, tag="nmu", name=f"nmu_{tt}")
            nc.vector.tensor_tensor(out=nmu, in0=mv[:, 0:1], in1=rstd,
                                    op=mybir.AluOpType.mult)
            nc.vector.tensor_scalar_mul(out=nmu, in0=nmu, scalar1=-1.0)
            # Normalized value in bf16: halves the SBUF read bandwidth of
            # the (bandwidth-bound) gamma/beta ops on the Pool engine.
            z_t = epi.tile([P, H], BF16, tag="z", name=f"z_{tt}")
            nc.scalar.activation(
                out=z_t, in_=x_t,
                func=mybir.ActivationFunctionType.Identity,
                bias=nmu, scale=rstd,
            )
            # gamma/beta alternate between the Pool and Vector engines by
            # tile parity so neither serializes the epilogue tail.
            eng = nc.gpsimd if tt % 2 == 0 else nc.vector
            eng.tensor_mul(out=y_t, in0=z_t, in1=gamma_b)
            eng.tensor_add(out=y_t, in0=y_t, in1=beta_b)
            nc.sync.dma_start(y[rs, :], y_t)

        with (
            tc.tile_pool(name="attn", bufs=1) as attn,
            tc.tile_pool(name="xp", bufs=1) as xp,
        ):
            Vp_sb = attn.tile([P, NKT, NH, HD + 1], F8)  # V' [tok, h, 65]
            nc.vector.memset(Vp_sb[:, :, :, HD : HD + 1], ONES_VAL)

            xT_sb = xp.tile([P, NJ, S], F8)
            xqT_sb = xp.tile([P, NJ, SQ], F8)
            bv_b = xp.tile([P, H], F32)

            attn_pools = (
                tc.tile_pool(name="kq", bufs=2),       # per-pair K.T/Q.T
                tc.tile_pool(name="wchunk", bufs=2),
                tc.tile_pool(name="expP", bufs=4),
                tc.tile_pool(name="rcpP", bufs=2),
                tc.tile_pool(name="psS", bufs=2, space="PSUM"),
                tc.tile_pool(name="psC", bufs=2, space="PSUM"),
                tc.tile_pool(name="wv_pool", bufs=1),
            )
            kq, wchunk, expP, rcpP, psS, psC, wv_pool = [
                p.__enter__() for p in attn_pools]

            # Input loads in waves. The DMA engines round-robin the queued
            # transfers ~one per engine at ~17GB/s each, so a tensor's load
            # latency is set by its strip count, and everything queued early
            # shares the aggregate bandwidth. Wave 1 is exactly what the
            # first score groups + V projection need, in fine strips; wave 2
            # is the rest of x; woT (not needed until the epilogue ~300us in)
            # is deferred into the pair-1 loop.
            wk0 = wchunk.tile([P, NJ, P], F8, tag="wk")
            wq0 = wchunk.tile([P, NJ, P], F8, tag="wq")
            nc.sync.dma_start(wk0, chunked(wkT)[:, :, 0:P])
            nc.sync.dma_start(wq0, chunked(wqT)[:, :, 0:P])
            wv_sb = wv_pool.tile([P, NJ, H], F8)
            cx = chunked(xT)
            cq = chunked(xqT)
            cv = chunked(wvT)
            for st in range(4):  # xT tokens 0-511 in 128-col strips
                sl = slice(st * P, (st + 1) * P)
                nc.sync.dma_start(xT_sb[:, :, sl], cx[:, :, sl])
            for st in range(4):  # xqT queries 0-511 in 128-col strips
                sl = slice(st * P, (st + 1) * P)
                nc.sync.dma_start(xqT_sb[:, :, sl], cq[:, :, sl])
            for st in range(4):  # wv in 256-feature strips
                sl = slice(st * 256, (st + 1) * 256)
                nc.sync.dma_start(wv_sb[:, :, sl], cv[:, :, sl])
            nc.sync.dma_start(bv_b, bcast(bv))
            # wave 2: remaining xT (tokens 512-2047) and xqT (512-1023)
            for st in range(6):
                sl = slice(512 + st * 256, 512 + (st + 1) * 256)
                nc.sync.dma_start(xT_sb[:, :, sl], cx[:, :, sl])
            for st in range(2):
                sl = slice(512 + st * 256, 512 + (st + 1) * 256)
                nc.sync.dma_start(xqT_sb[:, :, sl], cq[:, :, sl])
            nc.sync.dma_start(gamma_b, bcast(gamma))
            nc.sync.dma_start(beta_b, bcast(beta))

            # --- K/Q projection pieces for one head pair (fout chunk i) ---
            # DoubleRow: hidden-dim chunk pairs (2j, 2j+1) contract 256/pass.
            # Split into per-512-token pieces so they can interleave into the
            # exp-paced score loops (a monolithic projection would sit behind
            # ACT-paced score stalls in the PE FIFO).
            def kq_alloc(i, wkc=None, wqc=None):
                if wkc is None:
                    wkc = wchunk.tile([P, NJ, P], F8, tag="wk",
                                      name=f"wk_{i}")
                    wqc = wchunk.tile([P, NJ, P], F8, tag="wq",
                                      name=f"wq_{i}")
                    nc.sync.dma_start(
                        wkc, chunked(wkT)[:, :, i * P : (i + 1) * P])
                    nc.sync.dma_start(
                        wqc, chunked(wqT)[:, :, i * P : (i + 1) * P])
                KTt = kq.tile([P, S], F8, tag="KT", name=f"KT_{i}")
                QTt = kq.tile([P, SQ], F8, tag="QT", name=f"QT_{i}")
                return {"i": i, "wk": wkc, "wq": wqc, "KT": KTt, "QT": QTt}

            def k_piece(pr, t):
                ps = psProj.tile([P, 512], F32, tag="psProj",
                                 name=f"psK_{pr['i']}_{t}")
                for j in range(NJ // 2):
                    nc.tensor.matmul(
                        ps,
                        lhsT=pr["wk"][:, 2 * j : 2 * j + 2, :],
                        rhs=xT_sb[:, 2 * j : 2 * j + 2,
                                  t * 512 : (t + 1) * 512],
                        start=(j == 0),
                        stop=(j == NJ // 2 - 1),
                        perf_mode=DR,
                    )
                nc.vector.tensor_scalar_add(
                    out=pr["KT"][:, t * 512 : (t + 1) * 512],
                    in0=ps, scalar1=bk_sb[:, pr["i"] : pr["i"] + 1])

            def q_piece(pr, t):
                ps = psProj.tile([P, 512], F32, tag="psProj",
                                 name=f"psQ_{pr['i']}_{t}")
                for j in range(NJ // 2):
                    nc.tensor.matmul(
                        ps,
                        lhsT=pr["wq"][:, 2 * j : 2 * j + 2, :],
                        rhs=xqT_sb[:, 2 * j : 2 * j + 2,
                                   t * 512 : (t + 1) * 512],
                        start=(j == 0),
                        stop=(j == NJ // 2 - 1),
                        perf_mode=DR,
                    )
                nc.vector.tensor_scalar_add(
                    out=pr["QT"][:, t * 512 : (t + 1) * 512],
                    in0=ps, scalar1=bq_sb[:, pr["i"] : pr["i"] + 1])

            # Pair 0's projection runs upfront (nothing else to hide it in).
            kq0 = kq_alloc(0, wk0, wq0)
            for t in range(S // 512):
                k_piece(kq0, t)
            for t in range(SQ // 512):
                q_piece(kq0, t)

            # --- V projection (emitted per-tile; runs under pair 0's exp) ---
            def v_proj_tile(tt):
                for fc in range(2):
                    ps = psProj.tile([P, 512], F32, tag="psProj",
                                     name=f"psV_{tt}_{fc}")
                    for j in range(NJ // 2):
                        nc.tensor.matmul(
                            ps,
                            lhsT=xT_sb[:, 2 * j : 2 * j + 2,
                                       tt * P : (tt + 1) * P],
                            rhs=wv_sb[:, 2 * j : 2 * j + 2,
                                      fc * 512 : (fc + 1) * 512],
                            start=(j == 0),
                            stop=(j == NJ // 2 - 1),
                            perf_mode=DR,
                        )
                    nc.vector.tensor_add(
                        out=Vp_sb[:, tt, fc * 8 : (fc + 1) * 8, 0:HD],
                        in0=ps.rearrange("p (h d) -> p h d", d=HD),
                        in1=bv_b[:, fc * 512 : (fc + 1) * 512].rearrange(
                            "p (h d) -> p h d", d=HD
                        ),
                    )
                # Fold exp(mask[k]) into V' (incl. the ones column), so the
                # softmax needs no per-key-tile bias in its exp activation.
                nc.vector.tensor_scalar_mul(
                    out=Vp_sb[:, tt, :, :], in0=Vp_sb[:, tt, :, :],
                    scalar1=em_sb[:, tt : tt + 1])

            # --- ctx piece: one DoubleRow kt-pair accumulation, both heads ---
            def ctx_piece(jj, exp_ab, psc_of, u8):
                half, u = divmod(u8, NKT // 4)
                kt = half * (NKT // 2) + 2 * u
                for hh in (2 * jj, 2 * jj + 1):
                    nc.tensor.matmul(
                        psc_of[hh],
                        lhsT=Vp_sb[:, kt : kt + 2, hh, :],
                        rhs=exp_ab[half][:, 2 * u : 2 * u + 2, hh % 2, :],
                        start=(kt == 0),
                        stop=(kt == NKT - 2),
                        perf_mode=DR,
                    )

            def normalize(jj, qc, psc_of):
                qs = slice(qc * 512, (qc + 1) * 512)
                for hh in (2 * jj, 2 * jj + 1):
                    psc = psc_of[hh]
                    sume = rcpP.tile([1, 512], F32, tag="sume")
                    nc.vector.tensor_copy(out=sume, in_=psc[HD : HD + 1, :])
                    rcp = rcpP.tile([1, 512], F32, tag="rcp")
                    nc.vector.reciprocal_approx_fast(out=rcp, in_=sume)
                    rcpb = rcpP.tile([HD, 512], F32, tag="rcpb")
                    nc.gpsimd.partition_broadcast(rcpb, rcp)
                    po = 64 * (hh % 2)
                    nc.vector.tensor_mul(
                        out=ctxT_sb[po : po + 64, hh // 2, qs],
                        in0=psc[0:HD, :],
                        in1=rcpb,
                    )

            def alloc_psc(jj, qc):
                psc_of = {}
                for hh in (2 * jj, 2 * jj + 1):
                    psc_of[hh] = psC.tile([HD + 1, 512], F32, tag="psC",
                                          name=f"psc_{jj}_{qc}_{hh}")
                return psc_of

            # --- main attention loop over head pairs ---
            # The scalar engine's exp stream paces the loop, so every PE-side
            # job is split into pieces and interleaved at fixed kt slots of
            # the score loops (emission order = engine FIFO order, so a
            # monolithic job emitted after a score loop would execute after
            # it too, leaving the ACT idle at pair transitions):
            #   qc0 loop: prev pair's qc1-ctx pieces (odd kt), next pair's
            #             K-proj pieces (kt = 2 mod 4);
            #   qc1 loop: this pair's qc0-ctx pieces (odd kt), next pair's
            #             Q-proj pieces (kt 4, 12).
            #   Pair 0 instead carries the 16 V-proj tiles (even kt) and its
            #   proj pieces for pair 1 at odd kt slots.
            cur = kq0
            nxt = None
            pend = None  # (jj-1, exp_ab of qc1, psc_of) — ctx delayed here
            for jj in range(NPAIR):
                KTt, QTt = cur["KT"], cur["QT"]
                if jj == 1:
                    # deferred: epilogue weights, behind the startup waves
                    nc.sync.dma_start(woT_sb, chunked(woT))
                if jj + 1 < NPAIR:
                    nxt = kq_alloc(jj + 1)  # weight DMAs start now
                else:
                    nxt = None
                exp_of = {}
                psc_cur = {}
                for qc in range(NQC):
                    qs = slice(qc * 512, (qc + 1) * 512)
                    exp_a = expP.tile([P, NKT // 2, 2, 512], F8, tag="exp",
                                      name=f"exp_a_{jj}_{qc}")
                    exp_b = expP.tile([P, NKT // 2, 2, 512], F8, tag="exp",
                                      name=f"exp_b_{jj}_{qc}")
                    exp_of[qc] = [exp_a, exp_b]
                    # filler schedule: kt -> list of thunks
                    fillers = {}

                    def add(kt, fn):
                        fillers.setdefault(kt, []).append(fn)

                    if jj == 0:
                        for i2 in range(NKT // 2):
                            add(2 * i2, (lambda tt:
                                         (lambda: v_proj_tile(tt)))(
                                             qc * (NKT // 2) + i2))
                        if qc == 0 and nxt is not None:
                            for t in range(4):
                                add(2 * t + 1,
                                    (lambda t=t: k_piece(nxt, t)))
                            for t in range(2):
                                add(9 + 2 * t,
                                    (lambda t=t: q_piece(nxt, t)))
                        if qc == 1:
                            psc_cur[0] = alloc_psc(jj, 0)
                            for u8 in range(NKT // 2):
                                add(2 * u8 + 1,
                                    (lambda u8=u8: ctx_piece(
                                        jj, exp_of[0], psc_cur[0], u8)))
                    else:
                        if qc == 0:
                            if pend is not None:
                                pj, pexp, ppsc = pend
                                for u8 in range(NKT // 2):
                                    add(2 * u8 + 1,
                                        (lambda u8=u8: ctx_piece(
                                            pj, pexp, ppsc, u8)))
                            if nxt is not None:
                                for t in range(4):
                                    add(4 * t + 2,
                                        (lambda t=t: k_piece(nxt, t)))
                        else:
                            psc_cur[0] = alloc_psc(jj, 0)
                            for u8 in range(NKT // 2):
                                add(2 * u8 + 1,
                                    (lambda u8=u8: ctx_piece(
                                        jj, exp_of[0], psc_cur[0], u8)))
                            if nxt is not None:
                                for t in range(2):
                                    add(8 * t + 4,
                                        (lambda t=t: q_piece(nxt, t)))

                    for kt in range(NKT):
                        ks = slice(kt * P, (kt + 1) * P)
                        # Both heads of the pair score into ONE psum tile so
                        # their slots free together (row-group pair stays
                        # adjacent/concurrent) and one exp op drains both.
                        ps = psS.tile([P, 2, 512], F32, tag="psS")
                        nc.tensor.matmul(
                            ps[:, 0, :],
                            lhsT=KTt[0:64, ks], rhs=QTt[0:64, qs],
                            start=True, stop=True,
                        )
                        nc.tensor.matmul(
                            ps[:, 1, :],
                            lhsT=KTt[64:128, ks], rhs=QTt[64:128, qs],
                            start=True, stop=True,
                        )
                        nc.scalar.activation(
                            out=exp_of[qc][kt // (NKT // 2)][
                                :, kt % (NKT // 2), :, :],
                            in_=ps,
                            func=mybir.ActivationFunctionType.Exp,
                            bias=0.0, scale=EXP_SCALE,
                        )
                        for fn in fillers.get(kt, ()):
                            fn()

                    if qc == 0:
                        # prev pair's qc1 ctx just finished accumulating.
                        if pend is not None:
                            normalize(pend_jj, 1, pend[2])
                            pend = None
                    else:
                        normalize(jj, 0, psc_cur[0])

                if jj + 1 < NPAIR:
                    # ctx for this pair's qc1 interleaves into the next
                    # pair's qc0 score loop.
                    pend_jj = jj
                    pend = (jj, exp_of[1], alloc_psc(jj, 1))
                else:
                    # Last pair: epilogue for query rows 0-511 runs inside
                    # this pair's qc1 exp window (it needs only qc0's ctx),
                    # then qc1's ctx + normalize + remaining epilogue.
                    for tt in range(NTOK // 2):
                        emit_epilogue_tile(tt)
                    psc_last = alloc_psc(jj, 1)
                    for u8 in range(NKT // 2):
                        ctx_piece(jj, exp_of[1], psc_last, u8)
                    normalize(jj, 1, psc_last)
                cur = nxt

            for p in reversed(attn_pools):
                p.__exit__(None, None, None)

        # -------- epilogue tail: query rows 512-1023 --------
        for tt in range(NTOK // 2, NTOK):
            emit_epilogue_tile(tt)


def make_in_maps(hidden_states, attention_mask, wq, bq, wk, bk, wv, bv, wo,
                 bo, gamma, beta):
    """Shard/precompute host-side inputs for the 8 cores."""
    hs = np.asarray(hidden_states, dtype=np.float32)
    mask = np.asarray(attention_mask, dtype=np.float32).reshape(B, S)

    def chunk_cols(v, s):  # [H] -> [P, NJ]  (v[j*128+p] at [p, j])
        return np.ascontiguousarray(
            (np.asarray(v, np.float32) * s).reshape(NJ, P).T)

    def w8(w):  # [H, H] -> transposed, prescaled fp8
        return np.ascontiguousarray(
            np.asarray(w, np.float32).T * WS).astype(NPF8)

    bo32 = np.asarray(bo, np.float32)
    shared = {
        "wqT": w8(wq),
        "wkT": w8(wk),
        "wvT": w8(wv),
        "woT": w8(wo),
        "bq_c": chunk_cols(bq, QKV_S),
        "bk_c": chunk_cols(bk, QKV_S),
        "bv": np.asarray(bv, np.float32) * QKV_S,
        "gamma": np.asarray(gamma, np.float32).astype(ml_dtypes.bfloat16),
        "beta": np.asarray(beta, np.float32),
    }
    in_maps = []
    for c in range(N_CORES):
        b, half = divmod(c, 2)
        xb = hs[b]  # [S, H]
        xq = xb[half * SQ : (half + 1) * SQ]  # [SQ, H]
        m = {
            "xT": np.ascontiguousarray(xb.T * XS).astype(NPF8),
            "xqT": np.ascontiguousarray(xq.T * XS).astype(NPF8),
            # residual with bo pre-folded: the epilogue fuses
            # out*descale + (res+bo) in a single DVE op.
            "xres": np.ascontiguousarray(xq + bo32),
            "em_kt": np.ascontiguousarray(
                np.exp(mask[b]).reshape(NKT, P).T),
            **shared,
        }
        in_maps.append(m)
    return in_maps


_NC_CACHE = None


def kernel(**inputs):
    global _NC_CACHE
    from concourse.bass_utils import run_bass_kernel_spmd

    if _NC_CACHE is None:
        _NC_CACHE = build_program()
    nc = _NC_CACHE
    in_maps = make_in_maps(**inputs)
    res = run_bass_kernel_spmd(nc, in_maps, core_ids=list(range(N_CORES)))
    out = np.empty((B, S, H), np.float32)
    for c in range(N_CORES):
        b, half = divmod(c, 2)
        out[b, half * SQ : (half + 1) * SQ] = res.results[c]["y"]
    return out


# revision 28
# speedup vs baseline: 1.0294x; 1.0294x over previous
"""BERT attention block (QKV -> MHA -> output proj -> residual -> LayerNorm)
on 8 Trainium2 NeuronCores.

Sharding: data parallel over (batch, query-half). Core c handles batch b=c//2
and query rows [half*1024, (half+1)*1024) of that batch element (half=c%2).
Each core computes K/V for the full 2048-token sequence of its batch element
(duplicated across the 2 cores sharing a batch element), so no collectives
are needed. The per-core difference is entirely in the data (SPMD program).

All matmul operands are fp8 (e4m3, fp32 accumulation in PSUM), with DoubleRow
perf mode (two fp8 weights per PE cell -> contraction 256 per pass) on every
128-contraction matmul: the QKV/output projections pair adjacent hidden-dim
chunks, the ctx matmuls pair adjacent key tiles. Scores matmuls (contraction
64 per head) stay in normal mode with two heads packed via PE row groups.

The attention loop is paced by the scalar engine's exp throughput, so the
emission order keeps it saturated: per head pair, BOTH query chunks' scores
+exp run back-to-back (exp pool holds 4 half-tiles so the activation never
waits on ctx consumers), and the resulting ~33us exp window hides the V
projection (pair 0), the next pair's K/Q projection, both ctx accumulations,
and - on the last pair - the first half of the epilogue. The epilogue for
query rows 0-511 is emitted inside pair 7's qc=1 exp window (it only needs
qc=0's ctx columns), so only rows 512-1023 epilogue remains as tail.

fp8 scaling: x is prescaled x2 and weights x4 on the host so the weight
values clear e4m3's subnormal range. Q/K/V come out 8x true scale; scores
64x (folded into the exp scale); the softmax denominator's ones-column is
1/64 so ctx lands 512x its true value in fp8 (good range), and the output
projection descales by 1/2048 on the DVE. Softmax uses exp(s/8) with no max
subtraction (|s/8| is a few units at most for this distribution); the
attention mask folds in multiplicatively: V' rows (including the ones
column) are scaled by exp(mask[k]) after the V projection, so the exp
activation needs no per-key-tile bias and the denominator falls out of the
ctx matmul via V's scaled ones column (row 64 of the ctx accumulator).

The residual + LayerNorm path is fp32 end to end.
"""

import numpy as np
import ml_dtypes

import concourse.bass as bass
import concourse.mybir as mybir
import concourse.tile as tile
from concourse import bacc

# Problem constants (hardcoded per the harness contract).
B = 4
S = 2048
H = 1024
NH = 16
HD = 64
EPS = 1e-12
N_CORES = 8
SQ = 1024  # query rows per core
P = 128
NJ = H // P      # 8 hidden-dim chunks
NKT = S // P     # 16 key tiles
NQC = SQ // 512  # 2 query chunks of 512
NTOK = SQ // P   # 8 query-row tiles
NPAIR = NH // 2  # 8 head pairs

F8 = mybir.dt.float8e4
F32 = mybir.dt.float32
BF16 = mybir.dt.bfloat16
NPF8 = ml_dtypes.float8_e4m3
DR = mybir.MatmulPerfMode.DoubleRow

XS = 2.0                 # host prescale on x
WS = 4.0                 # host prescale on all four weight matrices
QKV_S = XS * WS          # q/k/v tiles are 8x true scale
EXP_SCALE = 0.125 / (QKV_S * QKV_S)   # exp(s_true/8) from 64x-scaled scores
ONES_VAL = 1.0 / 64.0    # denominator column value -> ctx stored 512x true
OUT_DESCALE = 1.0 / (64.0 * XS * WS * WS)  # after ctx @ woT


def build_program():
    nc = bacc.Bacc("TRN2", target_bir_lowering=False, debug=False)

    # DRAM layouts mirror the SBUF tile layouts exactly (host pre-arranges),
    # so every load is contiguous multi-KB runs per partition — fp8's 1B
    # elements make any strided pattern fall off the DMA efficiency cliff.
    xT = nc.dram_tensor("xT", [P, S // 512, NJ, 512], F8,
                        kind="ExternalInput").ap()
    xqT = nc.dram_tensor("xqT", [P, SQ // 512, NJ, 512], F8,
                         kind="ExternalInput").ap()
    xres = nc.dram_tensor("xres", [SQ, H], F32, kind="ExternalInput").ap()
    wqT = nc.dram_tensor("wqT", [P, NJ, NJ, P], F8,
                         kind="ExternalInput").ap()
    wkT = nc.dram_tensor("wkT", [P, NJ, NJ, P], F8,
                         kind="ExternalInput").ap()
    wvT = nc.dram_tensor("wvT", [P, NJ, H], F8, kind="ExternalInput").ap()
    woT = nc.dram_tensor("woT", [P, NJ, H], F8, kind="ExternalInput").ap()
    bq_c = nc.dram_tensor("bq_c", [P, NJ], F32, kind="ExternalInput").ap()
    bk_c = nc.dram_tensor("bk_c", [P, NJ], F32, kind="ExternalInput").ap()
    bv = nc.dram_tensor("bv", [H], F32, kind="ExternalInput").ap()
    gamma = nc.dram_tensor("gamma", [H], BF16, kind="ExternalInput").ap()
    beta = nc.dram_tensor("beta", [H], F32, kind="ExternalInput").ap()
    em_kt = nc.dram_tensor("em_kt", [P, NKT], F32, kind="ExternalInput").ap()
    y = nc.dram_tensor("y", [SQ, H], F32, kind="ExternalOutput").ap()

    with tile.TileContext(nc) as tc:
        _emit(tc, xT, xqT, xres, wqT, wkT, wvT, woT, bq_c, bk_c, bv,
              gamma, beta, em_kt, y)
    nc.compile()
    return nc


def _emit(tc, xT, xqT, xres, wqT, wkT, wvT, woT, bq_c, bk_c, bv, gamma,
          beta, em_kt, y):
    nc = tc.nc

    def bcast(v):  # [H] DRAM vector -> [P, H] partition-broadcast AP
        return bass.AP(tensor=v.tensor, offset=v.offset,
                       ap=[[0, P], list(v.ap[0])])

    with (
        tc.tile_pool(name="persist", bufs=1) as persist,
        tc.tile_pool(name="small", bufs=1) as small,
        tc.tile_pool(name="psProj", bufs=2, space="PSUM") as psProj,
        tc.tile_pool(name="epi", bufs=3) as epi,
        tc.tile_pool(name="stat", bufs=3) as stat,
    ):
        ctxT_sb = persist.tile([P, NJ, SQ], F8)  # 512x ctx.T [feat, tok]

        consts = small.tile([P, 2 * NJ + NKT + 1], F32)
        bq_sb = consts[:, 0:NJ]
        bk_sb = consts[:, NJ : 2 * NJ]
        em_sb = consts[:, 2 * NJ : 2 * NJ + NKT]
        eps_sb = consts[:, 2 * NJ + NKT :]
        nc.sync.dma_start(bq_sb, bq_c)
        nc.sync.dma_start(bk_sb, bk_c)
        nc.sync.dma_start(em_sb, em_kt)
        nc.vector.memset(eps_sb, EPS)
        gamma_b = small.tile([P, H], BF16)
        beta_b = small.tile([P, H], F32)
        woT_sb = persist.tile([P, NJ, H], F8)

        # ---- epilogue for one token tile: out proj + residual + LN ----
        def emit_epilogue_tile(tt):
            rs = slice(tt * P, (tt + 1) * P)
            x_t = epi.tile([P, H], F32, tag="x", name=f"x_{tt}")
            res_t = epi.tile([P, H], F32, tag="res", name=f"res_{tt}")
            y_t = epi.tile([P, H], F32, tag="y", name=f"y_{tt}")
            nc.sync.dma_start(res_t, xres[rs, :])
            for fc in range(2):
                fs = slice(fc * 512, (fc + 1) * 512)
                ps = psProj.tile([P, 512], F32, tag="psProj",
                                 name=f"psO_{tt}_{fc}")
                for j in range(NJ // 2):
                    nc.tensor.matmul(
                        ps,
                        lhsT=ctxT_sb[:, 2 * j : 2 * j + 2,
                                     tt * P : (tt + 1) * P],
                        rhs=woT_sb[:, 2 * j : 2 * j + 2, fs],
                        start=(j == 0),
                        stop=(j == NJ // 2 - 1),
                        perf_mode=DR,
                    )
                # bo is pre-folded into xres host-side, so one fused op:
                # x = ps*descale + (residual + bo).
                nc.vector.scalar_tensor_tensor(
                    out=x_t[:, fs], in0=ps, scalar=OUT_DESCALE,
                    in1=res_t[:, fs],
                    op0=mybir.AluOpType.mult, op1=mybir.AluOpType.add)
            st = stat.tile([P, 2, nc.vector.BN_STATS_DIM], F32, tag="st",
                           name=f"st_{tt}")
            mv = stat.tile([P, nc.vector.BN_AGGR_DIM], F32, tag="mv",
                           name=f"mv_{tt}")
            for g in range(2):
                nc.vector.bn_stats(out=st[:, g, :],
                                   in_=x_t[:, g * 512 : (g + 1) * 512])
            nc.vector.bn_aggr(out=mv, in_=st)
            sd = stat.tile([P, 1], F32, tag="sd", name=f"sd_{tt}")
            nc.scalar.activation(
                out=sd, in_=mv[:, 1:2],
                func=mybir.ActivationFunctionType.Sqrt,
                bias=eps_sb, scale=1.0,
            )
            rstd = stat.tile([P, 1], F32, tag="rstd", name=f"rstd_{tt}")
            nc.vector.reciprocal(rstd, sd)
            # x*rstd + (-mean*rstd) == (x - mean) * rstd, on the ACT engine.
            nmu = stat.tile([P, 1], F32, tag="nmu", name=f"nmu_{tt}")
            nc.vector.tensor_tensor(out=nmu, in0=mv[:, 0:1], in1=rstd,
                                    op=mybir.AluOpType.mult)
            nc.vector.tensor_scalar_mul(out=nmu, in0=nmu, scalar1=-1.0)
            # Normalized value in bf16: halves the SBUF read bandwidth of
            # the (bandwidth-bound) gamma/beta ops on the Pool engine.
            z_t = epi.tile([P, H], BF16, tag="z", name=f"z_{tt}")
            nc.scalar.activation(
                out=z_t, in_=x_t,
                func=mybir.ActivationFunctionType.Identity,
                bias=nmu, scale=rstd,
            )
            # gamma/beta alternate between the Pool and Vector engines by
            # tile parity so neither serializes the epilogue tail.
            eng = nc.gpsimd if tt % 2 == 0 else nc.vector
            eng.tensor_mul(out=y_t, in0=z_t, in1=gamma_b)
            eng.tensor_add(out=y_t, in0=y_t, in1=beta_b)
            nc.sync.dma_start(y[rs, :], y_t)

        with (
            tc.tile_pool(name="attn", bufs=1) as attn,
            tc.tile_pool(name="xp", bufs=1) as xp,
        ):
            Vp_sb = attn.tile([P, NKT, NH, HD + 1], F8)  # V' [tok, h, 65]
            nc.vector.memset(Vp_sb[:, :, :, HD : HD + 1], ONES_VAL)

            xT_sb = xp.tile([P, S // 512, NJ, 512], F8)
            xqT_sb = xp.tile([P, SQ // 512, NJ, 512], F8)
            bv_b = xp.tile([P, H], F32)

            attn_pools = (
                tc.tile_pool(name="kq", bufs=2),       # per-pair K.T/Q.T
                tc.tile_pool(name="wchunk", bufs=2),
                tc.tile_pool(name="expP", bufs=4),
                tc.tile_pool(name="rcpP", bufs=2),
                tc.tile_pool(name="psS", bufs=2, space="PSUM"),
                tc.tile_pool(name="psC", bufs=2, space="PSUM"),
                tc.tile_pool(name="wv_pool", bufs=1),
            )
            kq, wchunk, expP, rcpP, psS, psC, wv_pool = [
                p.__enter__() for p in attn_pools]

            # Input loads, critical-path first. Every transfer is contiguous
            # per partition (DRAM layouts mirror SBUF), so latency tracks
            # size. The first token chunks of xT/xqT plus pair 0's weights
            # unblock the score pipeline within a few microseconds; woT (not
            # needed until the epilogue ~300us in) is deferred to pair 1.
            wk0 = wchunk.tile([P, NJ, P], F8, tag="wk")
            wq0 = wchunk.tile([P, NJ, P], F8, tag="wq")
            nc.sync.dma_start(wk0, wkT[:, 0])
            nc.sync.dma_start(wq0, wqT[:, 0])
            wv_sb = wv_pool.tile([P, NJ, H], F8)
            nc.sync.dma_start(xT_sb[:, 0], xT[:, 0])
            nc.sync.dma_start(xqT_sb[:, 0], xqT[:, 0])
            nc.sync.dma_start(xT_sb[:, 1], xT[:, 1])
            nc.sync.dma_start(xqT_sb[:, 1], xqT[:, 1])
            nc.sync.dma_start(wv_sb, wvT)
            nc.sync.dma_start(bv_b, bcast(bv))
            nc.sync.dma_start(xT_sb[:, 2], xT[:, 2])
            nc.sync.dma_start(xT_sb[:, 3], xT[:, 3])
            nc.sync.dma_start(gamma_b, bcast(gamma))
            nc.sync.dma_start(beta_b, bcast(beta))

            # --- K/Q projection pieces for one head pair (fout chunk i) ---
            # DoubleRow: hidden-dim chunk pairs (2j, 2j+1) contract 256/pass.
            # Split into per-512-token pieces so they can interleave into the
            # exp-paced score loops (a monolithic projection would sit behind
            # ACT-paced score stalls in the PE FIFO).
            def kq_alloc(i, wkc=None, wqc=None):
                if wkc is None:
                    wkc = wchunk.tile([P, NJ, P], F8, tag="wk",
                                      name=f"wk_{i}")
                    wqc = wchunk.tile([P, NJ, P], F8, tag="wq",
                                      name=f"wq_{i}")
                    nc.sync.dma_start(wkc, wkT[:, i])
                    nc.sync.dma_start(wqc, wqT[:, i])
                KTt = kq.tile([P, S], F8, tag="KT", name=f"KT_{i}")
                QTt = kq.tile([P, SQ], F8, tag="QT", name=f"QT_{i}")
                return {"i": i, "wk": wkc, "wq": wqc, "KT": KTt, "QT": QTt}

            def k_piece(pr, t):
                ps = psProj.tile([P, 512], F32, tag="psProj",
                                 name=f"psK_{pr['i']}_{t}")
                for j in range(NJ // 2):
                    nc.tensor.matmul(
                        ps,
                        lhsT=pr["wk"][:, 2 * j : 2 * j + 2, :],
                        rhs=xT_sb[:, t, 2 * j : 2 * j + 2, :],
                        start=(j == 0),
                        stop=(j == NJ // 2 - 1),
                        perf_mode=DR,
                    )
                nc.vector.tensor_scalar_add(
                    out=pr["KT"][:, t * 512 : (t + 1) * 512],
                    in0=ps, scalar1=bk_sb[:, pr["i"] : pr["i"] + 1])

            def q_piece(pr, t):
                ps = psProj.tile([P, 512], F32, tag="psProj",
                                 name=f"psQ_{pr['i']}_{t}")
                for j in range(NJ // 2):
                    nc.tensor.matmul(
                        ps,
                        lhsT=pr["wq"][:, 2 * j : 2 * j + 2, :],
                        rhs=xqT_sb[:, t, 2 * j : 2 * j + 2, :],
                        start=(j == 0),
                        stop=(j == NJ // 2 - 1),
                        perf_mode=DR,
                    )
                nc.vector.tensor_scalar_add(
                    out=pr["QT"][:, t * 512 : (t + 1) * 512],
                    in0=ps, scalar1=bq_sb[:, pr["i"] : pr["i"] + 1])

            # Pair 0's projection runs upfront (nothing else to hide it in).
            kq0 = kq_alloc(0, wk0, wq0)
            for t in range(S // 512):
                k_piece(kq0, t)
            for t in range(SQ // 512):
                q_piece(kq0, t)

            # --- V projection (emitted per-tile; runs under pair 0's exp) ---
            def v_proj_tile(tt):
                for fc in range(2):
                    ps = psProj.tile([P, 512], F32, tag="psProj",
                                     name=f"psV_{tt}_{fc}")
                    for j in range(NJ // 2):
                        nc.tensor.matmul(
                            ps,
                            lhsT=xT_sb[:, tt // 4, 2 * j : 2 * j + 2,
                                       (tt % 4) * P : (tt % 4 + 1) * P],
                            rhs=wv_sb[:, 2 * j : 2 * j + 2,
                                      fc * 512 : (fc + 1) * 512],
                            start=(j == 0),
                            stop=(j == NJ // 2 - 1),
                            perf_mode=DR,
                        )
                    nc.vector.tensor_add(
                        out=Vp_sb[:, tt, fc * 8 : (fc + 1) * 8, 0:HD],
                        in0=ps.rearrange("p (h d) -> p h d", d=HD),
                        in1=bv_b[:, fc * 512 : (fc + 1) * 512].rearrange(
                            "p (h d) -> p h d", d=HD
                        ),
                    )
                # Fold exp(mask[k]) into V' (incl. the ones column), so the
                # softmax needs no per-key-tile bias in its exp activation.
                nc.vector.tensor_scalar_mul(
                    out=Vp_sb[:, tt, :, :], in0=Vp_sb[:, tt, :, :],
                    scalar1=em_sb[:, tt : tt + 1])

            # --- ctx piece: one DoubleRow kt-pair accumulation, both heads ---
            def ctx_piece(jj, exp_ab, psc_of, u8):
                half, u = divmod(u8, NKT // 4)
                kt = half * (NKT // 2) + 2 * u
                for hh in (2 * jj, 2 * jj + 1):
                    nc.tensor.matmul(
                        psc_of[hh],
                        lhsT=Vp_sb[:, kt : kt + 2, hh, :],
                        rhs=exp_ab[half][:, 2 * u : 2 * u + 2, hh % 2, :],
                        start=(kt == 0),
                        stop=(kt == NKT - 2),
                        perf_mode=DR,
                    )

            def normalize(jj, qc, psc_of):
                qs = slice(qc * 512, (qc + 1) * 512)
                for hh in (2 * jj, 2 * jj + 1):
                    psc = psc_of[hh]
                    sume = rcpP.tile([1, 512], F32, tag="sume")
                    nc.vector.tensor_copy(out=sume, in_=psc[HD : HD + 1, :])
                    rcp = rcpP.tile([1, 512], F32, tag="rcp")
                    nc.vector.reciprocal_approx_fast(out=rcp, in_=sume)
                    rcpb = rcpP.tile([HD, 512], F32, tag="rcpb")
                    nc.gpsimd.partition_broadcast(rcpb, rcp)
                    po = 64 * (hh % 2)
                    nc.vector.tensor_mul(
                        out=ctxT_sb[po : po + 64, hh // 2, qs],
                        in0=psc[0:HD, :],
                        in1=rcpb,
                    )

            def alloc_psc(jj, qc):
                psc_of = {}
                for hh in (2 * jj, 2 * jj + 1):
                    psc_of[hh] = psC.tile([HD + 1, 512], F32, tag="psC",
                                          name=f"psc_{jj}_{qc}_{hh}")
                return psc_of

            # --- main attention loop over head pairs ---
            # The scalar engine's exp stream paces the loop, so every PE-side
            # job is split into pieces and interleaved at fixed kt slots of
            # the score loops (emission order = engine FIFO order, so a
            # monolithic job emitted after a score loop would execute after
            # it too, leaving the ACT idle at pair transitions):
            #   qc0 loop: prev pair's qc1-ctx pieces (odd kt), next pair's
            #             K-proj pieces (kt = 2 mod 4);
            #   qc1 loop: this pair's qc0-ctx pieces (odd kt), next pair's
            #             Q-proj pieces (kt 4, 12).
            #   Pair 0 instead carries the 16 V-proj tiles (even kt) and its
            #   proj pieces for pair 1 at odd kt slots.
            cur = kq0
            nxt = None
            pend = None  # (jj-1, exp_ab of qc1, psc_of) — ctx delayed here
            for jj in range(NPAIR):
                KTt, QTt = cur["KT"], cur["QT"]
                if jj == 1:
                    # deferred: epilogue weights, behind the startup waves
                    nc.sync.dma_start(woT_sb, woT)
                if jj + 1 < NPAIR:
                    nxt = kq_alloc(jj + 1)  # weight DMAs start now
                else:
                    nxt = None
                exp_of = {}
                psc_cur = {}
                for qc in range(NQC):
                    qs = slice(qc * 512, (qc + 1) * 512)
                    exp_a = expP.tile([P, NKT // 2, 2, 512], F8, tag="exp",
                                      name=f"exp_a_{jj}_{qc}")
                    exp_b = expP.tile([P, NKT // 2, 2, 512], F8, tag="exp",
                                      name=f"exp_b_{jj}_{qc}")
                    exp_of[qc] = [exp_a, exp_b]
                    # filler schedule: kt -> list of thunks
                    fillers = {}

                    def add(kt, fn):
                        fillers.setdefault(kt, []).append(fn)

                    if jj == 0:
                        for i2 in range(NKT // 2):
                            add(2 * i2, (lambda tt:
                                         (lambda: v_proj_tile(tt)))(
                                             qc * (NKT // 2) + i2))
                        if qc == 0 and nxt is not None:
                            for t in range(4):
                                add(2 * t + 1,
                                    (lambda t=t: k_piece(nxt, t)))
                            for t in range(2):
                                add(9 + 2 * t,
                                    (lambda t=t: q_piece(nxt, t)))
                        if qc == 1:
                            psc_cur[0] = alloc_psc(jj, 0)
                            for u8 in range(NKT // 2):
                                add(2 * u8 + 1,
                                    (lambda u8=u8: ctx_piece(
                                        jj, exp_of[0], psc_cur[0], u8)))
                    else:
                        if qc == 0:
                            if pend is not None:
                                pj, pexp, ppsc = pend
                                for u8 in range(NKT // 2):
                                    add(2 * u8 + 1,
                                        (lambda u8=u8: ctx_piece(
                                            pj, pexp, ppsc, u8)))
                            if nxt is not None:
                                for t in range(4):
                                    add(4 * t + 2,
                                        (lambda t=t: k_piece(nxt, t)))
                        else:
                            psc_cur[0] = alloc_psc(jj, 0)
                            for u8 in range(NKT // 2):
                                add(2 * u8 + 1,
                                    (lambda u8=u8: ctx_piece(
                                        jj, exp_of[0], psc_cur[0], u8)))
                            if nxt is not None:
                                for t in range(2):
                                    add(8 * t + 4,
                                        (lambda t=t: q_piece(nxt, t)))

                    for kt in range(NKT):
                        ks = slice(kt * P, (kt + 1) * P)
                        # Both heads of the pair score into ONE psum tile so
                        # their slots free together (row-group pair stays
                        # adjacent/concurrent) and one exp op drains both.
                        ps = psS.tile([P, 2, 512], F32, tag="psS")
                        nc.tensor.matmul(
                            ps[:, 0, :],
                            lhsT=KTt[0:64, ks], rhs=QTt[0:64, qs],
                            start=True, stop=True,
                        )
                        nc.tensor.matmul(
                            ps[:, 1, :],
                            lhsT=KTt[64:128, ks], rhs=QTt[64:128, qs],
                            start=True, stop=True,
                        )
                        nc.scalar.activation(
                            out=exp_of[qc][kt // (NKT // 2)][
                                :, kt % (NKT // 2), :, :],
                            in_=ps,
                            func=mybir.ActivationFunctionType.Exp,
                            bias=0.0, scale=EXP_SCALE,
                        )
                        for fn in fillers.get(kt, ()):
                            fn()

                    if qc == 0:
                        # prev pair's qc1 ctx just finished accumulating.
                        if pend is not None:
                            normalize(pend_jj, 1, pend[2])
                            pend = None
                    else:
                        normalize(jj, 0, psc_cur[0])

                if jj + 1 < NPAIR:
                    # ctx for this pair's qc1 interleaves into the next
                    # pair's qc0 score loop.
                    pend_jj = jj
                    pend = (jj, exp_of[1], alloc_psc(jj, 1))
                else:
                    # Last pair: epilogue for query rows 0-511 runs inside
                    # this pair's qc1 exp window (it needs only qc0's ctx),
                    # then qc1's ctx + normalize + remaining epilogue.
                    for tt in range(NTOK // 2):
                        emit_epilogue_tile(tt)
                    psc_last = alloc_psc(jj, 1)
                    for u8 in range(NKT // 2):
                        ctx_piece(jj, exp_of[1], psc_last, u8)
                    normalize(jj, 1, psc_last)
                cur = nxt

            for p in reversed(attn_pools):
                p.__exit__(None, None, None)

        # -------- epilogue tail: query rows 512-1023 --------
        for tt in range(NTOK // 2, NTOK):
            emit_epilogue_tile(tt)


def make_in_maps(hidden_states, attention_mask, wq, bq, wk, bk, wv, bv, wo,
                 bo, gamma, beta):
    """Shard/precompute host-side inputs for the 8 cores."""
    hs = np.asarray(hidden_states, dtype=np.float32)
    mask = np.asarray(attention_mask, dtype=np.float32).reshape(B, S)

    def chunk_cols(v, s):  # [H] -> [P, NJ]  (v[j*128+p] at [p, j])
        return np.ascontiguousarray(
            (np.asarray(v, np.float32) * s).reshape(NJ, P).T)

    def wT8(w):  # [H, H] -> w.T prescaled fp8, still [H(in), H(out)]
        return (np.asarray(w, np.float32).T * WS).astype(NPF8)

    def feat_chunk(a):  # [H, N] -> [P, NJ, N]: a[j*128+p, n] at [p, j, n]
        Hh, N = a.shape
        return np.ascontiguousarray(a.reshape(NJ, P, N).transpose(1, 0, 2))

    def tok_chunk(a):  # [H, N] -> [P, N//512, NJ, 512]
        Hh, N = a.shape
        return np.ascontiguousarray(
            a.reshape(NJ, P, N // 512, 512).transpose(1, 2, 0, 3))

    def pair_chunk(a):  # [H, H] -> [P, NJ(pair), NJ, P]
        return np.ascontiguousarray(
            a.reshape(NJ, P, NJ, P).transpose(1, 2, 0, 3))

    bo32 = np.asarray(bo, np.float32)
    shared = {
        "wqT": pair_chunk(wT8(wq)),
        "wkT": pair_chunk(wT8(wk)),
        "wvT": feat_chunk(wT8(wv)),
        "woT": feat_chunk(wT8(wo)),
        "bq_c": chunk_cols(bq, QKV_S),
        "bk_c": chunk_cols(bk, QKV_S),
        "bv": np.asarray(bv, np.float32) * QKV_S,
        "gamma": np.asarray(gamma, np.float32).astype(ml_dtypes.bfloat16),
        "beta": np.asarray(beta, np.float32),
    }
    in_maps = []
    for c in range(N_CORES):
        b, half = divmod(c, 2)
        xb = hs[b]  # [S, H]
        xq = xb[half * SQ : (half + 1) * SQ]  # [SQ, H]
        m = {
            "xT": tok_chunk((xb.T * XS).astype(NPF8)),
            "xqT": tok_chunk((xq.T * XS).astype(NPF8)),
            # residual with bo pre-folded: the epilogue fuses
            # out*descale + (res+bo) in a single DVE op.
            "xres": np.ascontiguousarray(xq + bo32),
            "em_kt": np.ascontiguousarray(
                np.exp(mask[b]).reshape(NKT, P).T),
            **shared,
        }
        in_maps.append(m)
    return in_maps


_NC_CACHE = None


def kernel(**inputs):
    global _NC_CACHE
    from concourse.bass_utils import run_bass_kernel_spmd

    if _NC_CACHE is None:
        _NC_CACHE = build_program()
    nc = _NC_CACHE
    in_maps = make_in_maps(**inputs)
    res = run_bass_kernel_spmd(nc, in_maps, core_ids=list(range(N_CORES)))
    out = np.empty((B, S, H), np.float32)
    for c in range(N_CORES):
        b, half = divmod(c, 2)
        out[b, half * SQ : (half + 1) * SQ] = res.results[c]["y"]
    return out


# revision 35
# speedup vs baseline: 1.0326x; 1.0032x over previous
"""BERT attention block (QKV -> MHA -> output proj -> residual -> LayerNorm)
on 8 Trainium2 NeuronCores.

Sharding: data parallel over (batch, query-half). Core c handles batch b=c//2
and query rows [half*1024, (half+1)*1024) of that batch element (half=c%2).
Each core computes K/V for the full 2048-token sequence of its batch element
(duplicated across the 2 cores sharing a batch element), so no collectives
are needed. The per-core difference is entirely in the data (SPMD program).

All matmul operands are fp8 (e4m3, fp32 accumulation in PSUM), with DoubleRow
perf mode (two fp8 weights per PE cell -> contraction 256 per pass) on every
128-contraction matmul: the QKV/output projections pair adjacent hidden-dim
chunks, the ctx matmuls pair adjacent key tiles. Scores matmuls (contraction
64 per head) stay in normal mode with two heads packed via PE row groups.

The attention loop is paced by the scalar engine's exp throughput, so the
emission order keeps it saturated: per head pair, BOTH query chunks' scores
+exp run back-to-back (exp pool holds 4 half-tiles so the activation never
waits on ctx consumers), and the resulting ~33us exp window hides the V
projection (pair 0), the next pair's K/Q projection, both ctx accumulations,
and - on the last pair - the first half of the epilogue. The epilogue for
query rows 0-511 is emitted inside pair 7's qc=1 exp window (it only needs
qc=0's ctx columns), so only rows 512-1023 epilogue remains as tail.

fp8 scaling: x is prescaled x2 and weights x4 on the host so the weight
values clear e4m3's subnormal range. Q/K/V come out 8x true scale; scores
64x (folded into the exp scale); the softmax denominator's ones-column is
1/64 so ctx lands 512x its true value in fp8 (good range), and the output
projection descales by 1/2048 on the DVE. Softmax uses exp(s/8) with no max
subtraction (|s/8| is a few units at most for this distribution); the
attention mask folds in multiplicatively: V' rows (including the ones
column) are scaled by exp(mask[k]) after the V projection, so the exp
activation needs no per-key-tile bias and the denominator falls out of the
ctx matmul via V's scaled ones column (row 64 of the ctx accumulator).

The residual + LayerNorm path is fp32 end to end.
"""

import numpy as np
import ml_dtypes

import concourse.bass as bass
import concourse.mybir as mybir
import concourse.tile as tile
from concourse import bacc

# Problem constants (hardcoded per the harness contract).
B = 4
S = 2048
H = 1024
NH = 16
HD = 64
EPS = 1e-12
N_CORES = 8
SQ = 1024  # query rows per core
P = 128
NJ = H // P      # 8 hidden-dim chunks
NKT = S // P     # 16 key tiles
NQC = SQ // 512  # 2 query chunks of 512
NTOK = SQ // P   # 8 query-row tiles
NPAIR = NH // 2  # 8 head pairs

F8 = mybir.dt.float8e4
F32 = mybir.dt.float32
BF16 = mybir.dt.bfloat16
NPF8 = ml_dtypes.float8_e4m3
DR = mybir.MatmulPerfMode.DoubleRow

XS = 2.0                 # host prescale on x
WS = 4.0                 # host prescale on all four weight matrices
QKV_S = XS * WS          # q/k/v tiles are 8x true scale
EXP_SCALE = 0.125 / (QKV_S * QKV_S)   # exp(s_true/8) from 64x-scaled scores
ONES_VAL = 1.0 / 64.0    # denominator column value -> ctx stored 512x true
OUT_DESCALE = 1.0 / (64.0 * XS * WS * WS)  # after ctx @ woT

# Schraudolph fast-exp on the vector engine for a couple of key tiles per
# query chunk, relieving the scalar engine (the attention-loop pacer):
# exp(f) ~= bitcast_i32_to_f32(round(f * 2^23/ln2 + 127*2^23)). With the
# exact 127*2^23 bias the error is a one-sided sawtooth in [0, +6.1%],
# mean +4.08%; the mean is divided out of those key tiles' V'/em rows
# host-side, and the residue washes out over the softmax sum.
SCH_KT = (5, 11)         # key tiles offloaded to the DVE (qc=1 loop only)
SCH_A = float((1 << 23) / np.log(2) * EXP_SCALE)
SCH_B = float(127 * (1 << 23))
SCH_MEAN = 1.0408


def build_program():
    nc = bacc.Bacc("TRN2", target_bir_lowering=False, debug=False)

    # DRAM layouts mirror the SBUF tile layouts exactly (host pre-arranges),
    # so every load is contiguous multi-KB runs per partition — fp8's 1B
    # elements make any strided pattern fall off the DMA efficiency cliff.
    xT = nc.dram_tensor("xT", [P, S // 512, NJ, 512], F8,
                        kind="ExternalInput").ap()
    xqT = nc.dram_tensor("xqT", [P, SQ // 512, NJ, 512], F8,
                         kind="ExternalInput").ap()
    xres = nc.dram_tensor("xres", [SQ, H], F32, kind="ExternalInput").ap()
    wqT = nc.dram_tensor("wqT", [P, NJ, NJ, P], F8,
                         kind="ExternalInput").ap()
    wkT = nc.dram_tensor("wkT", [P, NJ, NJ, P], F8,
                         kind="ExternalInput").ap()
    wvT = nc.dram_tensor("wvT", [P, NJ, H], F8, kind="ExternalInput").ap()
    woT = nc.dram_tensor("woT", [P, NJ, H], F8, kind="ExternalInput").ap()
    bq_c = nc.dram_tensor("bq_c", [P, NJ], F32, kind="ExternalInput").ap()
    bk_c = nc.dram_tensor("bk_c", [P, NJ], F32, kind="ExternalInput").ap()
    bv = nc.dram_tensor("bv", [H], F32, kind="ExternalInput").ap()
    gamma = nc.dram_tensor("gamma", [H], BF16, kind="ExternalInput").ap()
    beta = nc.dram_tensor("beta", [H], F32, kind="ExternalInput").ap()
    em_kt = nc.dram_tensor("em_kt", [P, NKT], F32, kind="ExternalInput").ap()
    y = nc.dram_tensor("y", [SQ, H], F32, kind="ExternalOutput").ap()

    with tile.TileContext(nc) as tc:
        _emit(tc, xT, xqT, xres, wqT, wkT, wvT, woT, bq_c, bk_c, bv,
              gamma, beta, em_kt, y)
    nc.compile()
    return nc


def _emit(tc, xT, xqT, xres, wqT, wkT, wvT, woT, bq_c, bk_c, bv, gamma,
          beta, em_kt, y):
    nc = tc.nc

    def bcast(v):  # [H] DRAM vector -> [P, H] partition-broadcast AP
        return bass.AP(tensor=v.tensor, offset=v.offset,
                       ap=[[0, P], list(v.ap[0])])

    with (
        tc.tile_pool(name="persist", bufs=1) as persist,
        tc.tile_pool(name="small", bufs=1) as small,
        tc.tile_pool(name="psProj", bufs=2, space="PSUM") as psProj,
        tc.tile_pool(name="epi", bufs=3) as epi,
        tc.tile_pool(name="stat", bufs=3) as stat,
    ):
        ctxT_sb = persist.tile([P, NJ, SQ], F8)  # 512x ctx.T [feat, tok]

        consts = small.tile([P, 2 * NJ + NKT + 1], F32)
        bq_sb = consts[:, 0:NJ]
        bk_sb = consts[:, NJ : 2 * NJ]
        em_sb = consts[:, 2 * NJ : 2 * NJ + NKT]
        eps_sb = consts[:, 2 * NJ + NKT :]
        nc.sync.dma_start(bq_sb, bq_c)
        nc.sync.dma_start(bk_sb, bk_c)
        nc.sync.dma_start(em_sb, em_kt)
        nc.vector.memset(eps_sb, EPS)
        gamma_b = small.tile([P, H], BF16)
        beta_b = small.tile([P, H], F32)
        woT_sb = persist.tile([P, NJ, H], F8)

        # ---- epilogue for one token tile: out proj + residual + LN ----
        def emit_epilogue_tile(tt):
            rs = slice(tt * P, (tt + 1) * P)
            x_t = epi.tile([P, H], F32, tag="x", name=f"x_{tt}")
            res_t = epi.tile([P, H], F32, tag="res", name=f"res_{tt}")
            y_t = epi.tile([P, H], F32, tag="y", name=f"y_{tt}")
            nc.sync.dma_start(res_t, xres[rs, :])
            for fc in range(2):
                fs = slice(fc * 512, (fc + 1) * 512)
                ps = psProj.tile([P, 512], F32, tag="psProj",
                                 name=f"psO_{tt}_{fc}")
                for j in range(NJ // 2):
                    nc.tensor.matmul(
                        ps,
                        lhsT=ctxT_sb[:, 2 * j : 2 * j + 2,
                                     tt * P : (tt + 1) * P],
                        rhs=woT_sb[:, 2 * j : 2 * j + 2, fs],
                        start=(j == 0),
                        stop=(j == NJ // 2 - 1),
                        perf_mode=DR,
                    )
                # bo is pre-folded into xres host-side, so one fused op:
                # x = ps*descale + (residual + bo).
                nc.vector.scalar_tensor_tensor(
                    out=x_t[:, fs], in0=ps, scalar=OUT_DESCALE,
                    in1=res_t[:, fs],
                    op0=mybir.AluOpType.mult, op1=mybir.AluOpType.add)
            st = stat.tile([P, 2, nc.vector.BN_STATS_DIM], F32, tag="st",
                           name=f"st_{tt}")
            mv = stat.tile([P, nc.vector.BN_AGGR_DIM], F32, tag="mv",
                           name=f"mv_{tt}")
            for g in range(2):
                nc.vector.bn_stats(out=st[:, g, :],
                                   in_=x_t[:, g * 512 : (g + 1) * 512])
            nc.vector.bn_aggr(out=mv, in_=st)
            sd = stat.tile([P, 1], F32, tag="sd", name=f"sd_{tt}")
            nc.scalar.activation(
                out=sd, in_=mv[:, 1:2],
                func=mybir.ActivationFunctionType.Sqrt,
                bias=eps_sb, scale=1.0,
            )
            rstd = stat.tile([P, 1], F32, tag="rstd", name=f"rstd_{tt}")
            nc.vector.reciprocal(rstd, sd)
            # x*rstd + (-mean*rstd) == (x - mean) * rstd, on the ACT engine.
            nmu = stat.tile([P, 1], F32, tag="nmu", name=f"nmu_{tt}")
            nc.vector.tensor_tensor(out=nmu, in0=mv[:, 0:1], in1=rstd,
                                    op=mybir.AluOpType.mult)
            nc.vector.tensor_scalar_mul(out=nmu, in0=nmu, scalar1=-1.0)
            # Normalized value in bf16: halves the SBUF read bandwidth of
            # the (bandwidth-bound) gamma/beta ops on the Pool engine.
            z_t = epi.tile([P, H], BF16, tag="z", name=f"z_{tt}")
            nc.scalar.activation(
                out=z_t, in_=x_t,
                func=mybir.ActivationFunctionType.Identity,
                bias=nmu, scale=rstd,
            )
            # gamma/beta alternate between the Pool and Vector engines by
            # tile parity so neither serializes the epilogue tail.
            eng = nc.gpsimd if tt % 2 == 0 else nc.vector
            eng.tensor_mul(out=y_t, in0=z_t, in1=gamma_b)
            eng.tensor_add(out=y_t, in0=y_t, in1=beta_b)
            nc.sync.dma_start(y[rs, :], y_t)

        with (
            tc.tile_pool(name="attn", bufs=1) as attn,
            tc.tile_pool(name="xp", bufs=1) as xp,
        ):
            Vp_sb = attn.tile([P, NKT, NH, HD + 1], F8)  # V' [tok, h, 65]
            nc.vector.memset(Vp_sb[:, :, :, HD : HD + 1], ONES_VAL)

            xT_sb = xp.tile([P, S // 512, NJ, 512], F8)
            xqT_sb = xp.tile([P, SQ // 512, NJ, 512], F8)
            bv_b = xp.tile([P, H], F32)

            attn_pools = (
                tc.tile_pool(name="kq", bufs=2),       # per-pair K.T/Q.T
                tc.tile_pool(name="wchunk", bufs=2),
                tc.tile_pool(name="expP", bufs=4),
                tc.tile_pool(name="schP", bufs=2),
                tc.tile_pool(name="rcpP", bufs=2),
                tc.tile_pool(name="psS", bufs=2, space="PSUM"),
                tc.tile_pool(name="psC", bufs=2, space="PSUM"),
                tc.tile_pool(name="wv_pool", bufs=1),
            )
            kq, wchunk, expP, schP, rcpP, psS, psC, wv_pool = [
                p.__enter__() for p in attn_pools]

            # Input loads, critical-path first. Every transfer is contiguous
            # per partition (DRAM layouts mirror SBUF), so latency tracks
            # size. The first token chunks of xT/xqT plus pair 0's weights
            # unblock the score pipeline within a few microseconds; woT (not
            # needed until the epilogue ~300us in) is deferred to pair 1.
            wk0 = wchunk.tile([P, NJ, P], F8, tag="wk")
            wq0 = wchunk.tile([P, NJ, P], F8, tag="wq")
            nc.sync.dma_start(wk0, wkT[:, 0])
            nc.sync.dma_start(wq0, wqT[:, 0])
            wv_sb = wv_pool.tile([P, NJ, H], F8)
            nc.sync.dma_start(xT_sb[:, 0], xT[:, 0])
            nc.sync.dma_start(xqT_sb[:, 0], xqT[:, 0])
            nc.sync.dma_start(xT_sb[:, 1], xT[:, 1])
            nc.sync.dma_start(xqT_sb[:, 1], xqT[:, 1])
            nc.sync.dma_start(wv_sb, wvT)
            nc.sync.dma_start(bv_b, bcast(bv))
            nc.sync.dma_start(xT_sb[:, 2], xT[:, 2])
            nc.sync.dma_start(xT_sb[:, 3], xT[:, 3])
            nc.sync.dma_start(gamma_b, bcast(gamma))
            nc.sync.dma_start(beta_b, bcast(beta))

            # --- K/Q projection pieces for one head pair (fout chunk i) ---
            # DoubleRow: hidden-dim chunk pairs (2j, 2j+1) contract 256/pass.
            # Split into per-512-token pieces so they can interleave into the
            # exp-paced score loops (a monolithic projection would sit behind
            # ACT-paced score stalls in the PE FIFO).
            def kq_alloc(i, wkc=None, wqc=None):
                if wkc is None:
                    wkc = wchunk.tile([P, NJ, P], F8, tag="wk",
                                      name=f"wk_{i}")
                    wqc = wchunk.tile([P, NJ, P], F8, tag="wq",
                                      name=f"wq_{i}")
                    nc.sync.dma_start(wkc, wkT[:, i])
                    nc.sync.dma_start(wqc, wqT[:, i])
                KTt = kq.tile([P, S], F8, tag="KT", name=f"KT_{i}")
                QTt = kq.tile([P, SQ], F8, tag="QT", name=f"QT_{i}")
                return {"i": i, "wk": wkc, "wq": wqc, "KT": KTt, "QT": QTt}

            def k_piece(pr, t):
                ps = psProj.tile([P, 512], F32, tag="psProj",
                                 name=f"psK_{pr['i']}_{t}")
                for j in range(NJ // 2):
                    nc.tensor.matmul(
                        ps,
                        lhsT=pr["wk"][:, 2 * j : 2 * j + 2, :],
                        rhs=xT_sb[:, t, 2 * j : 2 * j + 2, :],
                        start=(j == 0),
                        stop=(j == NJ // 2 - 1),
                        perf_mode=DR,
                    )
                nc.vector.tensor_scalar_add(
                    out=pr["KT"][:, t * 512 : (t + 1) * 512],
                    in0=ps, scalar1=bk_sb[:, pr["i"] : pr["i"] + 1])

            def q_piece(pr, t):
                ps = psProj.tile([P, 512], F32, tag="psProj",
                                 name=f"psQ_{pr['i']}_{t}")
                for j in range(NJ // 2):
                    nc.tensor.matmul(
                        ps,
                        lhsT=pr["wq"][:, 2 * j : 2 * j + 2, :],
                        rhs=xqT_sb[:, t, 2 * j : 2 * j + 2, :],
                        start=(j == 0),
                        stop=(j == NJ // 2 - 1),
                        perf_mode=DR,
                    )
                nc.vector.tensor_scalar_add(
                    out=pr["QT"][:, t * 512 : (t + 1) * 512],
                    in0=ps, scalar1=bq_sb[:, pr["i"] : pr["i"] + 1])

            # Pair 0's projection runs upfront (nothing else to hide it in).
            # K/Q t=0 pieces first: the first score groups need only those,
            # so the exp pipeline starts ~4 cold-clock matmul groups sooner.
            kq0 = kq_alloc(0, wk0, wq0)
            k_piece(kq0, 0)
            q_piece(kq0, 0)
            k_piece(kq0, 1)
            q_piece(kq0, 1)
            k_piece(kq0, 2)
            k_piece(kq0, 3)

            # --- V projection (emitted per-tile; runs under pair 0's exp) ---
            def v_proj_tile(tt):
                for fc in range(2):
                    ps = psProj.tile([P, 512], F32, tag="psProj",
                                     name=f"psV_{tt}_{fc}")
                    for j in range(NJ // 2):
                        nc.tensor.matmul(
                            ps,
                            lhsT=xT_sb[:, tt // 4, 2 * j : 2 * j + 2,
                                       (tt % 4) * P : (tt % 4 + 1) * P],
                            rhs=wv_sb[:, 2 * j : 2 * j + 2,
                                      fc * 512 : (fc + 1) * 512],
                            start=(j == 0),
                            stop=(j == NJ // 2 - 1),
                            perf_mode=DR,
                        )
                    nc.vector.tensor_add(
                        out=Vp_sb[:, tt, fc * 8 : (fc + 1) * 8, 0:HD],
                        in0=ps.rearrange("p (h d) -> p h d", d=HD),
                        in1=bv_b[:, fc * 512 : (fc + 1) * 512].rearrange(
                            "p (h d) -> p h d", d=HD
                        ),
                    )
                # Fold exp(mask[k]) into V' (incl. the ones column), so the
                # softmax needs no per-key-tile bias in its exp activation.
                nc.vector.tensor_scalar_mul(
                    out=Vp_sb[:, tt, :, :], in0=Vp_sb[:, tt, :, :],
                    scalar1=em_sb[:, tt : tt + 1])

            # --- ctx piece: one DoubleRow kt-pair accumulation, both heads ---
            def ctx_piece(jj, exp_ab, psc_of, u8):
                half, u = divmod(u8, NKT // 4)
                kt = half * (NKT // 2) + 2 * u
                for hh in (2 * jj, 2 * jj + 1):
                    nc.tensor.matmul(
                        psc_of[hh],
                        lhsT=Vp_sb[:, kt : kt + 2, hh, :],
                        rhs=exp_ab[half][:, 2 * u : 2 * u + 2, hh % 2, :],
                        start=(kt == 0),
                        stop=(kt == NKT - 2),
                        perf_mode=DR,
                    )

            def normalize(jj, qc, psc_of):
                qs = slice(qc * 512, (qc + 1) * 512)
                for hh in (2 * jj, 2 * jj + 1):
                    psc = psc_of[hh]
                    sume = rcpP.tile([1, 512], F32, tag="sume")
                    nc.vector.tensor_copy(out=sume, in_=psc[HD : HD + 1, :])
                    rcp = rcpP.tile([1, 512], F32, tag="rcp")
                    nc.vector.reciprocal_approx_fast(out=rcp, in_=sume)
                    rcpb = rcpP.tile([HD, 512], F32, tag="rcpb")
                    nc.gpsimd.partition_broadcast(rcpb, rcp)
                    po = 64 * (hh % 2)
                    nc.vector.tensor_mul(
                        out=ctxT_sb[po : po + 64, hh // 2, qs],
                        in0=psc[0:HD, :],
                        in1=rcpb,
                    )

            def alloc_psc(jj, qc):
                psc_of = {}
                for hh in (2 * jj, 2 * jj + 1):
                    psc_of[hh] = psC.tile([HD + 1, 512], F32, tag="psC",
                                          name=f"psc_{jj}_{qc}_{hh}")
                return psc_of

            # --- main attention loop over head pairs ---
            # The scalar engine's exp stream paces the loop, so every PE-side
            # job is split into pieces and interleaved at fixed kt slots of
            # the score loops (emission order = engine FIFO order, so a
            # monolithic job emitted after a score loop would execute after
            # it too, leaving the ACT idle at pair transitions):
            #   qc0 loop: prev pair's qc1-ctx pieces (odd kt), next pair's
            #             K-proj pieces (kt = 2 mod 4);
            #   qc1 loop: this pair's qc0-ctx pieces (odd kt), next pair's
            #             Q-proj pieces (kt 4, 12).
            #   Pair 0 instead carries the 16 V-proj tiles (even kt) and its
            #   proj pieces for pair 1 at odd kt slots.
            cur = kq0
            nxt = None
            pend = None  # (jj-1, exp_ab of qc1, psc_of) — ctx delayed here
            for jj in range(NPAIR):
                KTt, QTt = cur["KT"], cur["QT"]
                if jj == 1:
                    # deferred: epilogue weights, behind the startup waves
                    nc.sync.dma_start(woT_sb, woT)
                if jj + 1 < NPAIR:
                    nxt = kq_alloc(jj + 1)  # weight DMAs start now
                else:
                    nxt = None
                exp_of = {}
                psc_cur = {}
                for qc in range(NQC):
                    qs = slice(qc * 512, (qc + 1) * 512)
                    exp_a = expP.tile([P, NKT // 2, 2, 512], F8, tag="exp",
                                      name=f"exp_a_{jj}_{qc}")
                    exp_b = expP.tile([P, NKT // 2, 2, 512], F8, tag="exp",
                                      name=f"exp_b_{jj}_{qc}")
                    exp_of[qc] = [exp_a, exp_b]
                    # filler schedule: kt -> list of thunks
                    fillers = {}

                    def add(kt, fn):
                        fillers.setdefault(kt, []).append(fn)

                    if jj == 0:
                        for i2 in range(NKT // 2):
                            add(2 * i2, (lambda tt:
                                         (lambda: v_proj_tile(tt)))(
                                             qc * (NKT // 2) + i2))
                        if qc == 0 and nxt is not None:
                            for t in range(4):
                                add(2 * t + 1,
                                    (lambda t=t: k_piece(nxt, t)))
                            for t in range(2):
                                add(9 + 2 * t,
                                    (lambda t=t: q_piece(nxt, t)))
                        if qc == 1:
                            psc_cur[0] = alloc_psc(jj, 0)
                            for u8 in range(NKT // 2):
                                add(2 * u8 + 1,
                                    (lambda u8=u8: ctx_piece(
                                        jj, exp_of[0], psc_cur[0], u8)))
                    else:
                        if qc == 0:
                            if pend is not None:
                                pj, pexp, ppsc = pend
                                for u8 in range(NKT // 2):
                                    add(2 * u8 + 1,
                                        (lambda u8=u8: ctx_piece(
                                            pj, pexp, ppsc, u8)))
                            if nxt is not None:
                                for t in range(4):
                                    add(4 * t + 2,
                                        (lambda t=t: k_piece(nxt, t)))
                        else:
                            psc_cur[0] = alloc_psc(jj, 0)
                            for u8 in range(NKT // 2):
                                add(2 * u8 + 1,
                                    (lambda u8=u8: ctx_piece(
                                        jj, exp_of[0], psc_cur[0], u8)))
                            if nxt is not None:
                                for t in range(2):
                                    add(8 * t + 4,
                                        (lambda t=t: q_piece(nxt, t)))

                    for kt in range(NKT):
                        ks = slice(kt * P, (kt + 1) * P)
                        # Both heads of the pair score into ONE psum tile so
                        # their slots free together (row-group pair stays
                        # adjacent/concurrent) and one exp op drains both.
                        ps = psS.tile([P, 2, 512], F32, tag="psS")
                        nc.tensor.matmul(
                            ps[:, 0, :],
                            lhsT=KTt[0:64, ks], rhs=QTt[0:64, qs],
                            start=True, stop=True,
                        )
                        nc.tensor.matmul(
                            ps[:, 1, :],
                            lhsT=KTt[64:128, ks], rhs=QTt[64:128, qs],
                            start=True, stop=True,
                        )
                        exp_out = exp_of[qc][kt // (NKT // 2)][
                            :, kt % (NKT // 2), :, :]
                        if qc == 1 and kt in SCH_KT:
                            # DVE fast-exp offload for this key tile.
                            ib = schP.tile([P, 2, 512], mybir.dt.int32,
                                           tag="ib", name=f"ib_{jj}_{kt}")
                            nc.vector.tensor_scalar(
                                ib, ps, SCH_A, SCH_B,
                                mybir.AluOpType.mult, mybir.AluOpType.add)
                            nc.vector.tensor_copy(
                                out=exp_out, in_=ib.bitcast(F32))
                        else:
                            nc.scalar.activation(
                                out=exp_out,
                                in_=ps,
                                func=mybir.ActivationFunctionType.Exp,
                                bias=0.0, scale=EXP_SCALE,
                            )
                        for fn in fillers.get(kt, ()):
                            fn()

                    if qc == 0:
                        # prev pair's qc1 ctx just finished accumulating.
                        if pend is not None:
                            normalize(pend_jj, 1, pend[2])
                            pend = None
                    else:
                        normalize(jj, 0, psc_cur[0])

                if jj + 1 < NPAIR:
                    # ctx for this pair's qc1 interleaves into the next
                    # pair's qc0 score loop.
                    pend_jj = jj
                    pend = (jj, exp_of[1], alloc_psc(jj, 1))
                else:
                    # Last pair: epilogue for query rows 0-511 runs inside
                    # this pair's qc1 exp window (it needs only qc0's ctx),
                    # then qc1's ctx + normalize + remaining epilogue.
                    for tt in range(NTOK // 2):
                        emit_epilogue_tile(tt)
                    psc_last = alloc_psc(jj, 1)
                    for u8 in range(NKT // 2):
                        ctx_piece(jj, exp_of[1], psc_last, u8)
                    normalize(jj, 1, psc_last)
                cur = nxt

            for p in reversed(attn_pools):
                p.__exit__(None, None, None)

        # -------- epilogue tail: query rows 512-1023 --------
        for tt in range(NTOK // 2, NTOK):
            emit_epilogue_tile(tt)


def make_in_maps(hidden_states, attention_mask, wq, bq, wk, bk, wv, bv, wo,
                 bo, gamma, beta):
    """Shard/precompute host-side inputs for the 8 cores."""
    hs = np.asarray(hidden_states, dtype=np.float32)
    mask = np.asarray(attention_mask, dtype=np.float32).reshape(B, S)

    def chunk_cols(v, s):  # [H] -> [P, NJ]  (v[j*128+p] at [p, j])
        return np.ascontiguousarray(
            (np.asarray(v, np.float32) * s).reshape(NJ, P).T)

    def wT8(w):  # [H, H] -> w.T prescaled fp8, still [H(in), H(out)]
        return (np.asarray(w, np.float32).T * WS).astype(NPF8)

    def feat_chunk(a):  # [H, N] -> [P, NJ, N]: a[j*128+p, n] at [p, j, n]
        Hh, N = a.shape
        return np.ascontiguousarray(a.reshape(NJ, P, N).transpose(1, 0, 2))

    def tok_chunk(a):  # [H, N] -> [P, N//512, NJ, 512]
        Hh, N = a.shape
        return np.ascontiguousarray(
            a.reshape(NJ, P, N // 512, 512).transpose(1, 2, 0, 3))

    def pair_chunk(a):  # [H, H] -> [P, NJ(pair), NJ, P]
        return np.ascontiguousarray(
            a.reshape(NJ, P, NJ, P).transpose(1, 2, 0, 3))

    def em_of(mrow):  # [S] mask -> [P, NKT] exp(mask), Schraudolph-adjusted
        em = np.exp(mrow).reshape(NKT, P).T.astype(np.float32)
        em[:, list(SCH_KT)] /= SCH_MEAN
        return em

    bo32 = np.asarray(bo, np.float32)
    shared = {
        "wqT": pair_chunk(wT8(wq)),
        "wkT": pair_chunk(wT8(wk)),
        "wvT": feat_chunk(wT8(wv)),
        "woT": feat_chunk(wT8(wo)),
        "bq_c": chunk_cols(bq, QKV_S),
        "bk_c": chunk_cols(bk, QKV_S),
        "bv": np.asarray(bv, np.float32) * QKV_S,
        "gamma": np.asarray(gamma, np.float32).astype(ml_dtypes.bfloat16),
        "beta": np.asarray(beta, np.float32),
    }
    in_maps = []
    for c in range(N_CORES):
        b, half = divmod(c, 2)
        xb = hs[b]  # [S, H]
        xq = xb[half * SQ : (half + 1) * SQ]  # [SQ, H]
        m = {
            "xT": tok_chunk((xb.T * XS).astype(NPF8)),
            "xqT": tok_chunk((xq.T * XS).astype(NPF8)),
            # residual with bo pre-folded: the epilogue fuses
            # out*descale + (res+bo) in a single DVE op.
            "xres": np.ascontiguousarray(xq + bo32),
            "em_kt": np.ascontiguousarray(em_of(mask[b])),
            **shared,
        }
        in_maps.append(m)
    return in_maps


_NC_CACHE = None


def kernel(**inputs):
    global _NC_CACHE
    from concourse.bass_utils import run_bass_kernel_spmd

    if _NC_CACHE is None:
        _NC_CACHE = build_program()
    nc = _NC_CACHE
    in_maps = make_in_maps(**inputs)
    res = run_bass_kernel_spmd(nc, in_maps, core_ids=list(range(N_CORES)))
    out = np.empty((B, S, H), np.float32)
    for c in range(N_CORES):
        b, half = divmod(c, 2)
        out[b, half * SQ : (half + 1) * SQ] = res.results[c]["y"]
    return out
